# revision 15
# baseline (speedup 1.0000x reference)
"""AQT int8 symmetric-quantized dot_general (bmk,kn->bmn) on 8 TRN2 NeuronCores.

Problem: lhs [2, 4096, 4096] f32, rhs [4096, 4096] f32.
  q_l, s_l = absmax-int8-quantize(lhs, axis=K)   (per-row scales)
  q_r, s_r = absmax-int8-quantize(rhs, axis=K)   (per-col scales)
  out = (q_l @ q_r) * s_l * s_r                  [2, 4096, 4096] f32

Sharding: 2 (batch) x 4 (N columns) grid over 8 cores; K replicated.
Each core computes an independent [4096, 1024] output block - no collectives.

Per-core kernel (Tile framework), v5.  Every engine queue is a FIFO, so a
not-yet-ready instruction head-of-line blocks everything behind it on that
queue; the emission order below is chosen so each queue's ops become ready
roughly in emission order:
  - rhs single HBM pass: 16 groups of [128, 2x1024] f32; scalar engine keeps
    a persistent SIGNED bf16 copy (sb); DVE runs max/min accumulators (bf16
    2x mode).  ALL 16 rhs DMA triggers are emitted on Sync before any
    transpose (a transpose emitted mid-stream blocks later rhs triggers on
    its qb dependency, starving the per-column amax that gates everything).
  - lhs prep is split into load (Sync DMA) / amax (DVE) / quantize (scalar
    act(lt*inv+MAGIC) in place then act(lt-MAGIC)->bf16) / xbar transpose
    (Sync).  m0/m1 load+amax ride the pass-1 DVE slack; m2-m4 amax ops are
    interspersed with the pass-2 production stream.
  - gpsimd does ONLY memset + partition_all_reduce (a dma_start on gpsimd
    forces an ~11us library reload before the allreduce).
  - pass 2 (SBUF only): ru_int16 = rne(sb * inv) (DVE 2x, RNE convert), one
    copyback into sb as bf16 - sb becomes q_r in place.
  - catch-up: m0/m1 matmuls are emitted kk-MAJOR so the PE consumes each
    quantized k-pair the moment DVE produces it.
  - remaining m-tiles panel-major, prepping 3-4 ahead; epilogue
    (psum * s_l) * s_r in one DVE op; DMA out f32.
"""

import numpy as np

import concourse.bass as bass
import concourse.mybir as mybir
import concourse.tile as tile
from concourse import bacc, bass_isa
from concourse.bass import ts
from concourse.bass_utils import run_bass_kernel_spmd

MAGIC = 12582912.0  # 1.5 * 2**23: fp32 add => round-half-even to integer

B, M, K, N = 2, 4096, 4096, 4096
GRID_B, GRID_N = 2, 4  # 8 cores
M_LOC, N_LOC = M, N // GRID_N


def build_nc(m_loc=M_LOC, k=K, n_loc=N_LOC, panel=512):
    f32, bf16, i16 = mybir.dt.float32, mybir.dt.bfloat16, mybir.dt.int16
    mult, add = mybir.AluOpType.mult, mybir.AluOpType.add
    vmax, vmin = mybir.AluOpType.max, mybir.AluOpType.min
    nk, nm, npan = k // 128, m_loc // 128, n_loc // panel
    ng = nk // 2  # rhs DMA groups of 2 k-tiles
    n_catch = 2  # m-tiles consumed kk-major during rhs quantize production
    nc = bacc.Bacc("TRN2", target_bir_lowering=False, debug=False)
    lhs_d = nc.dram_tensor("lhs", [m_loc, k], f32, kind="ExternalInput")
    rhs_d = nc.dram_tensor("rhs", [k, n_loc], f32, kind="ExternalInput")
    out_d = nc.dram_tensor("out", [m_loc, n_loc], f32, kind="ExternalOutput")

    with tile.TileContext(nc) as tc:
        with (
            tc.tile_pool(name="rstat", bufs=1) as rstatp,
            tc.tile_pool(name="rio", bufs=3) as riop,
            tc.tile_pool(name="sb", bufs=1) as sbp,
            tc.tile_pool(name="rtmp", bufs=1) as rtmpp,
            tc.tile_pool(name="lio", bufs=3) as liop,
            tc.tile_pool(name="lqb", bufs=2) as lqbp,
            tc.tile_pool(name="lqt", bufs=3) as lqtp,
            tc.tile_pool(name="lstat", bufs=8) as lstatp,
            tc.tile_pool(name="eo", bufs=2) as eop,
            tc.tile_pool(name="pout", bufs=6, space="PSUM") as poutp,
        ):
            # ---------- rhs pass 1: stream + signed bf16 copy + max/min ----
            accA = rstatp.tile([128, 2 * n_loc], bf16, tag="accA")
            accB = rstatp.tile([128, 2 * n_loc], bf16, tag="accB")
            nc.gpsimd.memset(accA[:], 0.0)
            nc.gpsimd.memset(accB[:], 0.0)

            sb_tiles = []

            def rhs_group(g):
                rt = riop.tile([128, 2 * n_loc], f32, tag="rt")
                nc.sync.dma_start(
                    rt[:].rearrange("p (t n) -> p t n", t=2),
                    rhs_d[ts(g, 256), :].rearrange("(t p) n -> p t n", p=128),
                )
                sb = sbp.tile([128, 2 * n_loc], bf16, tag=f"sb{g}")
                nc.scalar.copy(sb[:], rt[:])
                nc.vector.tensor_tensor(accA[:], accA[:], sb[:], op=vmax)
                nc.vector.tensor_tensor(accB[:], accB[:], sb[:], op=vmin)
                sb_tiles.append(sb)

            # ---- lhs m-tile prep, split per engine ----
            lt_tiles, ls_tiles, qb_tiles, prepped = {}, {}, {}, {}

            def prep_load(mi):  # Sync: DMA trigger
                lt = liop.tile([128, k], f32, tag="lt")
                nc.sync.dma_start(lt[:], lhs_d[ts(mi, 128), :])
                lt_tiles[mi] = lt

            def prep_amax(mi):  # DVE: amax + scales
                lt = lt_tiles[mi]
                am = lstatp.tile([128, 1], f32, tag="am")
                nc.vector.tensor_reduce(
                    am[:],
                    lt[:],
                    axis=mybir.AxisListType.X,
                    op=vmax,
                    apply_absolute_value=True,
                )
                inv_l = lstatp.tile([128, 1], f32, tag="invl")
                nc.vector.reciprocal(inv_l[:], am[:])
                nc.vector.tensor_scalar_mul(inv_l[:], inv_l[:], 127.0)
                s_l = lstatp.tile([128, 1], f32, tag="sl")
                nc.vector.tensor_scalar_mul(s_l[:], am[:], 1.0 / 127.0)
                ls_tiles[mi] = (inv_l, s_l)

            def prep_act(mi):  # scalar: quantize via MAGIC, f32 -> bf16
                lt = lt_tiles.pop(mi)
                inv_l, _ = ls_tiles[mi]
                nc.scalar.activation(
                    lt[:], lt[:], mybir.ActivationFunctionType.Copy,
                    bias=MAGIC, scale=inv_l[:],
                )
                qb = lqbp.tile([128, k], bf16, tag="qb")
                nc.scalar.activation(
                    qb[:], lt[:], mybir.ActivationFunctionType.Copy, bias=-MAGIC
                )
                qb_tiles[mi] = qb

            def prep_xpose(mi):  # Sync: one xbar transpose for all nk blocks
                qb = qb_tiles.pop(mi)
                _, s_l = ls_tiles.pop(mi)
                qT = lqtp.tile([128, k], bf16, tag="qT")
                # out[p, b, f] = qb[f, b*128 + p]
                nc.sync.dma_start_transpose(
                    qT[:].rearrange("p (b f) -> p b f", f=128), qb[:]
                )
                prepped[mi] = (qT, s_l)

            def prep_all(mi):
                prep_amax(mi)
                prep_act(mi)
                prep_xpose(mi)

            # rhs stream with m0/m1 loads + amax in the DVE slack; no
            # transposes are emitted on Sync until all rt triggers are out.
            for g in range(ng):
                rhs_group(g)
                if g == 8:
                    prep_load(0)
                elif g == 12:
                    prep_amax(0)
            prep_act(0)
            prep_load(1)
            prep_xpose(0)

            # ---------- fold halves -> amax, allreduce, scales -------------
            nc.vector.tensor_tensor(
                accA[:, 0:n_loc], accA[:, 0:n_loc], accA[:, n_loc : 2 * n_loc],
                op=vmax,
            )
            nc.vector.tensor_tensor(
                accB[:, 0:n_loc], accB[:, 0:n_loc], accB[:, n_loc : 2 * n_loc],
                op=vmin,
            )
            accm = rstatp.tile([128, n_loc], f32, tag="accm")
            nc.vector.scalar_tensor_tensor(
                accm[:], accB[:, 0:n_loc], -1.0, accA[:, 0:n_loc],
                op0=mult, op1=vmax,
            )
            amax_r = rstatp.tile([128, n_loc], f32, tag="amax_r")
            nc.gpsimd.partition_all_reduce(
                amax_r[:], accm[:], channels=128, reduce_op=bass_isa.ReduceOp.absmax
            )
            inv_r = rstatp.tile([128, n_loc], f32, tag="accm")  # reuse slot
            nc.vector.reciprocal_approx_fast(inv_r[:], amax_r[:])
            inv_rb = rstatp.tile([128, n_loc], bf16, tag="inv_rb")
            nc.vector.tensor_scalar_mul(inv_rb[:], inv_r[:], 127.0)
            s_r = rstatp.tile([128, n_loc], f32, tag="s_r")
            nc.vector.tensor_scalar_mul(s_r[:], amax_r[:], 1.0 / 127.0)
            inv_rb2 = (
                inv_rb[:]
                .rearrange("p (o n) -> p o n", o=1)
                .broadcast_to((128, 2, n_loc))
            )
            prep_amax(1)
            prep_act(1)
            prep_xpose(1)

            # ---------- rhs pass 2 (SBUF only): quantize sb in place -------
            # m2-m4 amax reduces are spread through the production stream.
            for g in range(ng):
                sb = sb_tiles[g]
                ru = rtmpp.tile([128, 2 * n_loc], i16, tag="ru")
                nc.vector.tensor_tensor(
                    ru[:].rearrange("p (o n) -> p o n", o=2),
                    sb[:].rearrange("p (o n) -> p o n", o=2),
                    inv_rb2,
                    op=mult,
                )
                nc.vector.tensor_scalar_mul(sb[:], ru[:], 1.0)
                if g == 0:
                    prep_load(2)
                elif g == 3:
                    prep_amax(2)
                    prep_act(2)
                    prep_load(3)
                elif g == 7:
                    prep_xpose(2)
                    prep_amax(3)
                    prep_act(3)
                    prep_load(4)
                elif g == 11:
                    prep_amax(4)
                    prep_act(4)
                    prep_load(5)
                elif g == 13:
                    prep_xpose(3)

            prep_xpose(4)

            def qr_ap(kk):  # quantized rhs k-tile kk as [128, n_loc] bf16
                return sb_tiles[kk // 2][:, (kk % 2) * n_loc : (kk % 2 + 1) * n_loc]

            def epilogue(mi, p, po, s_l):
                eo = eop.tile([128, panel], f32, tag="eo")
                nc.vector.scalar_tensor_tensor(
                    eo[:], po[:], s_l[:], s_r[:, ts(p, panel)], op0=mult, op1=mult
                )
                nc.scalar.dma_start(out_d[ts(mi, 128), ts(p, panel)], eo[:])

            def mm_mtile(mi, qT, s_l):
                for p in range(npan):
                    po = poutp.tile([128, panel], f32, tag="po")
                    for kk in range(nk):
                        nc.tensor.matmul(
                            po[:],
                            qT[:, ts(kk, 128)],
                            qr_ap(kk)[:, ts(p, panel)],
                            start=(kk == 0),
                            stop=(kk == nk - 1),
                        )
                    epilogue(mi, p, po, s_l)

            # ---------- catch-up: m-tiles 0..n_catch-1 kk-major ------------
            catch_po = {}
            for m in range(n_catch):
                for p in range(npan):
                    po_c = poutp.tile([128, panel], f32, tag="po")
                    catch_po[(m, p)] = po_c
            for kk in range(nk):
                for m in range(n_catch):
                    qT, _ = prepped[m]
                    for p in range(npan):
                        nc.tensor.matmul(
                            catch_po[(m, p)][:],
                            qT[:, ts(kk, 128)],
                            qr_ap(kk)[:, ts(p, panel)],
                            start=(kk == 0),
                            stop=(kk == nk - 1),
                        )
            for m in range(n_catch):
                _, s_l = prepped.pop(m)
                for p in range(npan):
                    epilogue(m, p, catch_po[(m, p)], s_l)

            # ---------- steady m-tile loop, loads 4 / full preps 3 ahead ---
            for mi in range(n_catch, nm):
                for j in range(mi + 1, min(mi + 5, nm)):
                    if j not in lt_tiles and j not in prepped and j not in ls_tiles:
                        prep_load(j)
                for j in range(mi + 1, min(mi + 4, nm)):
                    if j in lt_tiles and j not in ls_tiles and j not in prepped:
                        prep_all(j)
                if mi not in prepped:
                    prep_all(mi)
                qT, s_l = prepped.pop(mi)
                mm_mtile(mi, qT, s_l)

    nc.compile()
    return nc


def run_shards(nc, lhs_shards, rhs_shards, trace=False, **kw):
    in_maps = [
        {"lhs": np.ascontiguousarray(l), "rhs": np.ascontiguousarray(r)}
        for l, r in zip(lhs_shards, rhs_shards)
    ]
    return run_bass_kernel_spmd(
        nc, in_maps, core_ids=list(range(len(in_maps))), trace=trace, **kw
    )


_NC_CACHE = {}


def get_full_nc():
    if "nc" not in _NC_CACHE:
        _NC_CACHE["nc"] = build_nc()
    return _NC_CACHE["nc"]


def kernel(lhs, rhs):
    lhs = np.ascontiguousarray(np.asarray(lhs, dtype=np.float32))
    rhs = np.ascontiguousarray(np.asarray(rhs, dtype=np.float32))
    assert lhs.shape == (B, M, K) and rhs.shape == (K, N)
    nc = get_full_nc()
    lhs_shards, rhs_shards = [], []
    for c in range(8):
        pi, qi = c // GRID_N, c % GRID_N
        lhs_shards.append(lhs[pi])
        rhs_shards.append(rhs[:, qi * N_LOC : (qi + 1) * N_LOC])
    res = run_shards(nc, lhs_shards, rhs_shards)
    out = np.empty((B, M, N), np.float32)
    for c in range(8):
        pi, qi = c // GRID_N, c % GRID_N
        out[pi, :, qi * N_LOC : (qi + 1) * N_LOC] = res.results[c]["out"]
    return out


if __name__ == "__main__":
    rng = np.random.default_rng(0)
    lhs = rng.standard_normal((B, M, K), dtype=np.float32)
    rhs = rng.standard_normal((K, N), dtype=np.float32)
    out = kernel(lhs=lhs, rhs=rhs)
    print("kernel output:", out.shape, out.dtype)


# revision 16
# speedup vs baseline: 1.0265x; 1.0265x over previous
"""AQT int8 symmetric-quantized dot_general (bmk,kn->bmn) on 8 TRN2 NeuronCores.

Problem: lhs [2, 4096, 4096] f32, rhs [4096, 4096] f32.
  q_l, s_l = absmax-int8-quantize(lhs, axis=K)   (per-row scales)
  q_r, s_r = absmax-int8-quantize(rhs, axis=K)   (per-col scales)
  out = (q_l @ q_r) * s_l * s_r                  [2, 4096, 4096] f32

Sharding: 2 (batch) x 4 (N columns) grid over 8 cores; K replicated.
Each core computes an independent [4096, 1024] output block - no collectives.

Per-core kernel (Tile framework), v7.  Engine queues are FIFOs (a not-ready
instruction head-of-line blocks its queue), so emission order tracks the
intended execution order, and the lhs prep chain is split into k-HALVES to
halve its latency (amax -> act(h1) -> transpose(h1) can feed the PE while
act(h2)/transpose(h2) are still running):
  - rhs single HBM pass: 16 groups of [128, 2x1024] f32; scalar keeps a
    persistent SIGNED bf16 copy (sb); DVE runs max/min accs (bf16 2x mode).
    All 16 rhs DMA triggers are emitted on Sync before any transpose (the
    per-column amax gates on the LAST group).
  - gpsimd does ONLY memset + partition_all_reduce (a dma_start on gpsimd
    forces an ~11us library reload before the allreduce).
  - pass 2 (SBUF only): ru_int16 = rne(sb * inv) (DVE 2x, RNE convert), one
    copyback into sb as bf16 - sb becomes q_r in place.
  - lhs per m-tile: DVE amax; scalar act(lt*inv+MAGIC) in place + act(-MAGIC)
    -> bf16, in two k-halves; two half xbar DMA-transposes put K on
    partitions.  m0/m1 ride the pass-1 slack; m2-m4 are spread through the
    pass-2 production stream.
  - catch-up: m0/m1 matmuls emitted kk-MAJOR so the PE consumes each
    quantized k-pair the moment DVE produces it.
  - remaining m-tiles panel-major, prepping 3-4 ahead; epilogue
    (psum * s_l) * s_r in one DVE op; DMA out f32.
"""

import numpy as np

import concourse.bass as bass
import concourse.mybir as mybir
import concourse.tile as tile
from concourse import bacc, bass_isa
from concourse.bass import ts
from concourse.bass_utils import run_bass_kernel_spmd

MAGIC = 12582912.0  # 1.5 * 2**23: fp32 add => round-half-even to integer

B, M, K, N = 2, 4096, 4096, 4096
GRID_B, GRID_N = 2, 4  # 8 cores
M_LOC, N_LOC = M, N // GRID_N


def build_nc(m_loc=M_LOC, k=K, n_loc=N_LOC, panel=512):
    f32, bf16, i16 = mybir.dt.float32, mybir.dt.bfloat16, mybir.dt.int16
    mult, add = mybir.AluOpType.mult, mybir.AluOpType.add
    vmax, vmin = mybir.AluOpType.max, mybir.AluOpType.min
    nk, nm, npan = k // 128, m_loc // 128, n_loc // panel
    ng = nk // 2  # rhs DMA groups of 2 k-tiles
    n_catch = 2  # m-tiles consumed kk-major during rhs quantize production
    kh = k // 2  # lhs prep works in k-halves to shorten the chain latency
    nc = bacc.Bacc("TRN2", target_bir_lowering=False, debug=False)
    lhs_d = nc.dram_tensor("lhs", [m_loc, k], f32, kind="ExternalInput")
    rhs_d = nc.dram_tensor("rhs", [k, n_loc], f32, kind="ExternalInput")
    out_d = nc.dram_tensor("out", [m_loc, n_loc], f32, kind="ExternalOutput")

    with tile.TileContext(nc) as tc:
        with (
            tc.tile_pool(name="rstat", bufs=1) as rstatp,
            tc.tile_pool(name="rio", bufs=2) as riop,
            tc.tile_pool(name="sb", bufs=1) as sbp,
            tc.tile_pool(name="rtmp", bufs=1) as rtmpp,
            tc.tile_pool(name="lio", bufs=3) as liop,
            tc.tile_pool(name="lqb", bufs=2) as lqbp,
            tc.tile_pool(name="lqt", bufs=4) as lqtp,
            tc.tile_pool(name="lstat", bufs=8) as lstatp,
            tc.tile_pool(name="eo", bufs=2) as eop,
            tc.tile_pool(name="pout", bufs=6, space="PSUM") as poutp,
        ):
            # ---------- rhs pass 1: stream + signed bf16 copy + max/min ----
            accA = rstatp.tile([128, 2 * n_loc], bf16, tag="accA")
            accB = rstatp.tile([128, 2 * n_loc], bf16, tag="accB")
            nc.gpsimd.memset(accA[:], 0.0)
            nc.gpsimd.memset(accB[:], 0.0)

            sb_tiles = []

            def rhs_group(g):
                rt = riop.tile([128, 2 * n_loc], f32, tag="rt")
                nc.sync.dma_start(
                    rt[:].rearrange("p (t n) -> p t n", t=2),
                    rhs_d[ts(g, 256), :].rearrange("(t p) n -> p t n", p=128),
                )
                sb = sbp.tile([128, 2 * n_loc], bf16, tag=f"sb{g}")
                nc.scalar.copy(sb[:], rt[:])
                nc.vector.tensor_tensor(accA[:], accA[:], sb[:], op=vmax)
                nc.vector.tensor_tensor(accB[:], accB[:], sb[:], op=vmin)
                sb_tiles.append(sb)

            # ---- lhs m-tile prep, split per engine and per k-half ----
            lt_tiles, ls_tiles, qb_tiles, prepped = {}, {}, {}, {}

            def prep_load(mi):  # Sync: DMA trigger
                lt = liop.tile([128, k], f32, tag="lt")
                nc.sync.dma_start(lt[:], lhs_d[ts(mi, 128), :])
                lt_tiles[mi] = lt

            def prep_amax(mi):  # DVE: amax + scales
                lt = lt_tiles[mi]
                am = lstatp.tile([128, 1], f32, tag="am")
                nc.vector.tensor_reduce(
                    am[:],
                    lt[:],
                    axis=mybir.AxisListType.X,
                    op=vmax,
                    apply_absolute_value=True,
                )
                inv_l = lstatp.tile([128, 1], f32, tag="invl")
                nc.vector.reciprocal(inv_l[:], am[:])
                nc.vector.tensor_scalar_mul(inv_l[:], inv_l[:], 127.0)
                s_l = lstatp.tile([128, 1], f32, tag="sl")
                nc.vector.tensor_scalar_mul(s_l[:], am[:], 1.0 / 127.0)
                ls_tiles[mi] = (inv_l, s_l)

            def prep_act(mi):  # scalar: quantize via MAGIC, f32 -> bf16
                lt = lt_tiles.pop(mi)
                inv_l, _ = ls_tiles[mi]
                qb = lqbp.tile([128, k], bf16, tag="qb")
                for h in range(2):
                    sl = slice(h * kh, (h + 1) * kh)
                    nc.scalar.activation(
                        lt[:, sl], lt[:, sl], mybir.ActivationFunctionType.Copy,
                        bias=MAGIC, scale=inv_l[:],
                    )
                    nc.scalar.activation(
                        qb[:, sl], lt[:, sl],
                        mybir.ActivationFunctionType.Copy, bias=-MAGIC,
                    )
                qb_tiles[mi] = qb

            def prep_xpose(mi):  # Sync: two half xbar transposes
                qb = qb_tiles.pop(mi)
                _, s_l = ls_tiles.pop(mi)
                qT = lqtp.tile([128, k], bf16, tag="qT")
                # out[p, b, f] = qb[f, b*128 + p]
                for h in range(2):
                    sl = slice(h * kh, (h + 1) * kh)
                    nc.sync.dma_start_transpose(
                        qT[:, sl].rearrange("p (b f) -> p b f", f=128),
                        qb[:, sl],
                    )
                prepped[mi] = (qT, s_l)

            def prep_all(mi):
                prep_amax(mi)
                prep_act(mi)
                prep_xpose(mi)

            # rhs stream; m0/m1 load + amax ride the pass-1 DVE slack, m0's
            # quantize runs in the scalar slack near the stream tail.
            for g in range(ng):
                rhs_group(g)
                if g == 5:
                    prep_load(0)
                elif g == 8:
                    prep_load(1)
                elif g == 11:
                    prep_amax(0)
                elif g == 13:
                    prep_act(0)
                elif g == 14:
                    prep_amax(1)
            prep_act(1)
            prep_load(2)
            prep_load(3)
            prep_xpose(0)
            prep_xpose(1)

            # ---------- fold halves -> amax, allreduce, scales -------------
            nc.vector.tensor_tensor(
                accA[:, 0:n_loc], accA[:, 0:n_loc], accA[:, n_loc : 2 * n_loc],
                op=vmax,
            )
            nc.vector.tensor_tensor(
                accB[:, 0:n_loc], accB[:, 0:n_loc], accB[:, n_loc : 2 * n_loc],
                op=vmin,
            )
            accm = rstatp.tile([128, n_loc], f32, tag="accm")
            nc.vector.scalar_tensor_tensor(
                accm[:], accB[:, 0:n_loc], -1.0, accA[:, 0:n_loc],
                op0=mult, op1=vmax,
            )
            amax_r = rstatp.tile([128, n_loc], f32, tag="amax_r")
            nc.gpsimd.partition_all_reduce(
                amax_r[:], accm[:], channels=128, reduce_op=bass_isa.ReduceOp.absmax
            )
            inv_r = rstatp.tile([128, n_loc], f32, tag="accm")  # reuse slot
            nc.vector.reciprocal_approx_fast(inv_r[:], amax_r[:])
            inv_rb = rstatp.tile([128, n_loc], bf16, tag="inv_rb")
            nc.vector.tensor_scalar_mul(inv_rb[:], inv_r[:], 127.0)
            s_r = rstatp.tile([128, n_loc], f32, tag="s_r")
            nc.vector.tensor_scalar_mul(s_r[:], amax_r[:], 1.0 / 127.0)
            inv_rb2 = (
                inv_rb[:]
                .rearrange("p (o n) -> p o n", o=1)
                .broadcast_to((128, 2, n_loc))
            )

            # ---------- rhs pass 2 (SBUF only): quantize sb in place -------
            # m2-m4 prep stages are spread through the production stream.
            for g in range(ng):
                sb = sb_tiles[g]
                ru = rtmpp.tile([128, 2 * n_loc], i16, tag="ru")
                nc.vector.tensor_tensor(
                    ru[:].rearrange("p (o n) -> p o n", o=2),
                    sb[:].rearrange("p (o n) -> p o n", o=2),
                    inv_rb2,
                    op=mult,
                )
                nc.vector.tensor_scalar_mul(sb[:], ru[:], 1.0)
                if g == 0:
                    prep_amax(2)
                    prep_act(2)
                elif g == 4:
                    prep_xpose(2)
                    prep_load(4)
                elif g == 6:
                    prep_amax(3)
                    prep_act(3)
                elif g == 9:
                    prep_xpose(3)
                    prep_load(5)
                elif g == 11:
                    prep_amax(4)
                    prep_act(4)
                elif g == 14:
                    prep_xpose(4)

            def qr_ap(kk):  # quantized rhs k-tile kk as [128, n_loc] bf16
                return sb_tiles[kk // 2][:, (kk % 2) * n_loc : (kk % 2 + 1) * n_loc]

            def epilogue(mi, p, po, s_l):
                eo = eop.tile([128, panel], f32, tag="eo")
                nc.vector.scalar_tensor_tensor(
                    eo[:], po[:], s_l[:], s_r[:, ts(p, panel)], op0=mult, op1=mult
                )
                nc.scalar.dma_start(out_d[ts(mi, 128), ts(p, panel)], eo[:])

            def mm_mtile(mi, qT, s_l):
                for p in range(npan):
                    po = poutp.tile([128, panel], f32, tag="po")
                    for kk in range(nk):
                        nc.tensor.matmul(
                            po[:],
                            qT[:, ts(kk, 128)],
                            qr_ap(kk)[:, ts(p, panel)],
                            start=(kk == 0),
                            stop=(kk == nk - 1),
                        )
                    epilogue(mi, p, po, s_l)

            # ---------- catch-up: m-tiles 0..n_catch-1 kk-major ------------
            catch_po = {}
            for m in range(n_catch):
                for p in range(npan):
                    po_c = poutp.tile([128, panel], f32, tag="po")
                    catch_po[(m, p)] = po_c
            for kk in range(nk):
                for m in range(n_catch):
                    qT, _ = prepped[m]
                    for p in range(npan):
                        nc.tensor.matmul(
                            catch_po[(m, p)][:],
                            qT[:, ts(kk, 128)],
                            qr_ap(kk)[:, ts(p, panel)],
                            start=(kk == 0),
                            stop=(kk == nk - 1),
                        )
            for m in range(n_catch):
                _, s_l = prepped.pop(m)
                for p in range(npan):
                    epilogue(m, p, catch_po[(m, p)], s_l)

            # ---------- steady m-tile loop, loads 4 / full preps 3 ahead ---
            for mi in range(n_catch, nm):
                for j in range(mi + 1, min(mi + 5, nm)):
                    if j not in lt_tiles and j not in prepped and j not in ls_tiles:
                        prep_load(j)
                for j in range(mi + 1, min(mi + 4, nm)):
                    if j in lt_tiles and j not in ls_tiles and j not in prepped:
                        prep_all(j)
                if mi not in prepped:
                    prep_all(mi)
                qT, s_l = prepped.pop(mi)
                mm_mtile(mi, qT, s_l)

    nc.compile()
    return nc


def run_shards(nc, lhs_shards, rhs_shards, trace=False, **kw):
    in_maps = [
        {"lhs": np.ascontiguousarray(l), "rhs": np.ascontiguousarray(r)}
        for l, r in zip(lhs_shards, rhs_shards)
    ]
    return run_bass_kernel_spmd(
        nc, in_maps, core_ids=list(range(len(in_maps))), trace=trace, **kw
    )


_NC_CACHE = {}


def get_full_nc():
    if "nc" not in _NC_CACHE:
        _NC_CACHE["nc"] = build_nc()
    return _NC_CACHE["nc"]


def kernel(lhs, rhs):
    lhs = np.ascontiguousarray(np.asarray(lhs, dtype=np.float32))
    rhs = np.ascontiguousarray(np.asarray(rhs, dtype=np.float32))
    assert lhs.shape == (B, M, K) and rhs.shape == (K, N)
    nc = get_full_nc()
    lhs_shards, rhs_shards = [], []
    for c in range(8):
        pi, qi = c // GRID_N, c % GRID_N
        lhs_shards.append(lhs[pi])
        rhs_shards.append(rhs[:, qi * N_LOC : (qi + 1) * N_LOC])
    res = run_shards(nc, lhs_shards, rhs_shards)
    out = np.empty((B, M, N), np.float32)
    for c in range(8):
        pi, qi = c // GRID_N, c % GRID_N
        out[pi, :, qi * N_LOC : (qi + 1) * N_LOC] = res.results[c]["out"]
    return out


if __name__ == "__main__":
    rng = np.random.default_rng(0)
    lhs = rng.standard_normal((B, M, K), dtype=np.float32)
    rhs = rng.standard_normal((K, N), dtype=np.float32)
    out = kernel(lhs=lhs, rhs=rhs)
    print("kernel output:", out.shape, out.dtype)
